# revision 56
# baseline (speedup 1.0000x reference)
"""Trainium2 Bass kernel for gathered-row MLP decode matmul.

out[b, 0, r] = sum_d x[b, 0, d] * weight[indices[r], d]

Active path (kernel() -> run_full(mode="hi8") -> _build_hi8): dedup the
indices on the host, shard the ~3629 unique rows across 8 cores (~454
each, padded to 512 = 4x128 with duplicate indices). The weight is cast
to fp8 e3m4 on the host with a x128 prescale (4-bit mantissa; end-to-end
scale-rel error ~1.2e-2 against the 2e-2 gate), halving HBM gather
traffic vs fp16. The transpose-gather DMA works at 16-bit granularity,
so fp8 values land as interleaved even/odd pairs
[p, f, r, e] = w8[idx_r, 2*(f*128+p)+e]; the matmuls use stride-2 lhsT
views with an x layout reordered to match (x/128 folded in, so no
epilogue scale). Critical-path engineering, all verified against the
TimelineSim cost model and real-HW runs:
  - the gather-index DMA is issued before the TileContext entry barrier
    (raw SBUF tensor + manual sem), and split from the scatter-iota half
    so desc-gen starts as early as possible;
  - the first two 128-row gathers go through prepare_only + trigger_dma,
    skipping the 650ns DGE->DMA handoff so the five gather transfers run
    back-to-back on the DMA engines from ~4.0us;
  - the last chunk is fetched as 10/6 column pieces so only a 546ns
    transfer plus 12 matmuls are exposed on the tail;
  - outputs accumulate in PSUM [rows, B] fp32, are copied to SBUF (last
    chunk via the faster DVE path), and leave via two prepared SWDGE
    scatters with disjoint out_dram slices (avoids a false WAW edge),
    triggered right after the copies land.
Tile's deferred-dep machinery does not bind producers emitted after a
prep and never increments the DMASW lane sems of gen_mode==1 preps, so
_build_hi8 post-processes the scheduled BIR: trigger placeholder waits
are re-pointed at the gating copy's engine-tick sem, and orphaned DMASW
lane waits at the preps' user DMA sems. The host transposes/assembles
per-core outputs and inverse-maps duplicates back to the original
4403-index order. The older fp16 path (_build_hi) is kept as a fallback
for index distributions that don't fit 464 rows/core.
"""
import os
import sys
from contextlib import ExitStack

sys.path.insert(0, "/opt/trn_rl_repo")
os.environ.setdefault("MYCRO_LOCAL_CACHE", "1")

import numpy as np

D_FF = 11008
D_MODEL = 4096
R_TOTAL = 4403
B = 32
NCORES = 8
P = 128
KT = D_MODEL // P          # 32 contraction tiles (fp16 path)
KT2 = D_MODEL // 256       # 16 u16-granularity tiles (fp8 path)
NV8 = 512                  # padded per-core row count for the fp8 path
W_SCALE = 128.0            # host weight prescale for e3m4 range

_cache = {}
_FILLS = (0, 0, 0, 0)


def _build_hi8(gbufs=4):
    """fp8(e3m4) weight-stationary kernel, 512 rows/core in 4x128 chunks."""
    key = ("hi8", gbufs)
    if key in _cache:
        return _cache[key]
    from concourse import bacc, mybir, tile

    f32 = mybir.dt.float32
    f16 = mybir.dt.float16
    f8 = mybir.dt.float8e3
    i16 = mybir.dt.int16

    nch = NV8 // 128  # 4 chunks
    nv16 = NV8 // 16  # 32 idx cols

    nc = bacc.Bacc(
        "TRN2",
        target_bir_lowering=False,
        debug=False,
        enable_asserts=False,
        num_swdge_queues=4,
    )
    whi_dram = nc.dram_tensor("whi", [D_FF, D_MODEL], f8, kind="ExternalInput").ap()
    xq_dram = nc.dram_tensor("xq", [P, KT2 * 2 * B], f16, kind="ExternalInput").ap()
    # first nv16 cols: gather indices; next nv16: iota rows for the scatter
    idx_dram = nc.dram_tensor("idx", [P, 2 * nv16], i16, kind="ExternalInput").ap()
    # 64-wide rows so the scatter elem is 256B (cols 32..63 are zero pad)
    out_dram = nc.dram_tensor("out", [NV8, 2 * B], f32, kind="ExternalOutput").ap()

    # idx DMA issued BEFORE the TileContext entry barrier (~640ns earlier):
    # raw SBUF tensor + manual completion sem; every SWDGE desc-gen that
    # reads it carries an explicit wait
    idx_sb_h = nc.alloc_sbuf_tensor("idx_sb8", [P, 2 * nv16], i16)
    idx_sem = nc.alloc_semaphore("idx_dma")
    nc.gpsimd.sem_clear(idx_sem)
    # NOTE: the wrapped idx layout must cover all 128 partitions -- each of
    # the 8 GPSIMD Q7 cores reads its own 16-partition replica on real HW.
    # Gather-index half first (smaller transfer -> earlier desc-gen); the
    # scatter-iota half follows on its own sem (preps run much later).
    idx2_sem = nc.alloc_semaphore("idx2_dma")
    nc.gpsimd.sem_clear(idx2_sem)
    nc.sync.dma_start(
        idx_sb_h.ap()[:, :nv16], idx_dram[:, :nv16]
    ).then_inc(idx_sem, 16)
    nc.sync.dma_start(
        idx_sb_h.ap()[:, nv16:], idx_dram[:, nv16:]
    ).then_inc(idx2_sem, 16)
    idx_sb = idx_sb_h.ap()

    with tile.TileContext(nc) as tc, ExitStack() as ctx:
        consts = ctx.enter_context(tc.tile_pool(name="consts", bufs=1))
        whi_pool = ctx.enter_context(tc.tile_pool(name="whiT", bufs=gbufs))
        psum = ctx.enter_context(tc.tile_pool(name="psum", bufs=4, space="PSUM"))

        xq_sb = consts.tile([P, KT2 * 2 * B], f16)
        nc.sync.dma_start(xq_sb[:], xq_dram)
        obs = consts.tile([P, nch, 2 * B], f32)
        nc.vector.memset(obs[:], 0)

        # all gather desc-gens first so the Pool engine pipelines ahead of
        # the serialized DMA transfers. The FIRST gather goes through
        # prepare_only + trigger: a triggered transfer skips the 650ns
        # DGE->DMA handoff delay, so the transfer block starts earlier.
        # chunks 0 and 1 via prepare_only + trigger: a triggered transfer
        # skips the 650ns DGE->DMA handoff, so g0 fires right after its
        # desc-gen and g1 packs immediately behind it
        gsems = [nc.alloc_semaphore(f"g{c}_dma") for c in range(2)]
        for s in gsems:
            nc.gpsimd.sem_clear(s)
        wts = []
        for c in range(nch):
            wt = whi_pool.tile([P, KT2 * P * 2], f8, tag=f"wt{c}")
            idx_c = idx_sb[:, c * 8 : (c + 1) * 8]
            if c < 2:
                nc.gpsimd.dma_gather(
                    out_ap=wt[:].rearrange("p (a r) -> p a r", a=KT2 * 2, r=P),
                    in_ap=whi_dram,
                    idxs_ap=idx_c,
                    num_idxs=P,
                    num_idxs_reg=P,
                    elem_size=D_MODEL,
                    transpose=True,
                    prepare_only=True,
                    sem=gsems[c],
                    queue_num=3,
                ).wait_op(idx_sem, 0, "sem-ge")
                if c == 0:
                    # fires g0 only (g1's prep not yet recorded)
                    nc.gpsimd.trigger_dma(count=None, queue_num=3)
            elif c == nch - 1:
                # last chunk in a 10/6 column split: the final exposed
                # transfer is small and its desc-gen still fits the chain
                pieces = [(0, 10), (10, 6)]
                for f0, nf in pieces:
                    nc.gpsimd.dma_gather(
                        out_ap=wt[:].rearrange(
                            "p (a r) -> p a r", a=KT2 * 2, r=P
                        )[:, f0 * 2 : (f0 + nf) * 2, :],
                        in_ap=whi_dram[:, f0 * 256 : (f0 + nf) * 256],
                        idxs_ap=idx_c,
                        num_idxs=P,
                        num_idxs_reg=P,
                        elem_size=nf * 256,
                        elem_step=D_MODEL,
                        transpose=True,
                    ).wait_op(idx_sem, 0, "sem-ge")
            else:
                nc.gpsimd.dma_gather(
                    out_ap=wt[:].rearrange("p (a r) -> p a r", a=KT2 * 2, r=P),
                    in_ap=whi_dram,
                    idxs_ap=idx_c,
                    num_idxs=P,
                    num_idxs_reg=P,
                    elem_size=D_MODEL,
                    transpose=True,
                ).wait_op(idx_sem, 0, "sem-ge")
            wts.append(wt)

        # prepared output scatters: desc-gen runs now (Pool is idle once the
        # gather desc-gens finish); triggers fire after the PSUM copies land.
        # Tile's deferred-dep machinery doesn't bind producers emitted after
        # the prep, so the copy->trigger ordering is a manual sem protocol.
        semA = nc.alloc_semaphore("outA_dma")
        semB = nc.alloc_semaphore("outB_dma")
        cp_sem = nc.alloc_semaphore("cp_done")  # placeholder, rewritten below
        for s in (semA, semB):
            nc.gpsimd.sem_clear(s)
        # standalone Pool wait: the scatter preps' desc-gen reads the iota
        # half of idx (separate DMA); Pool is in-order so one wait covers both
        nc.gpsimd.wait_ge(idx2_sem, 0)
        # disjoint out_dram slices so Tile sees no WAW between the two
        # scatters (a full-tensor out_ap serializes trigger B behind
        # scatter A's DMA completion); B's iota values are slice-relative
        prepA = nc.gpsimd.dma_scatter_add(
            out_ap=out_dram[: (nch - 1) * P, :],
            in_ap=obs[:, : nch - 1, :],
            idxs_ap=idx_sb[:, nv16 : nv16 + (nch - 1) * 8],
            num_idxs=(nch - 1) * P,
            num_idxs_reg=(nch - 1) * P,
            elem_size=2 * B,
            prepare_only=True,
            sem=semA,
            queue_num=1,
        ).wait_op(idx_sem, 0, "sem-ge")
        prepB = nc.gpsimd.dma_scatter_add(
            out_ap=out_dram[(nch - 1) * P :, :],
            in_ap=obs[:, nch - 1 :, :],
            idxs_ap=idx_sb[:, nv16 + (nch - 1) * 8 : nv16 + (nch - 1) * 8 + 5],
            num_idxs=80,
            num_idxs_reg=80,
            elem_size=2 * B,
            prepare_only=True,
            sem=semB,
            queue_num=2,
        ).wait_op(idx_sem, 0, "sem-ge")
        nc.gpsimd.trigger_dma(count=None, queue_num=3)  # fires g1

        xq4 = xq_sb[:].rearrange("p (f e b) -> p f e b", f=KT2, e=2, b=B)
        fill_pool = ctx.enter_context(
            tc.tile_pool(name="fillps", bufs=1, space="PSUM")
        )
        fill_ps = fill_pool.tile([P, B], f32, name="fill_ps")
        w40 = wts[0][:].rearrange("p (f i e) -> p f i e", f=KT2, i=P, e=2)

        def _fill(n):
            # p-state fillers: keep the PE busy across data-wait gaps so the
            # clock ramp survives to the tail matmuls (chunk0's tile is
            # available data; results discarded)
            for _ in range(n):
                nc.tensor.matmul(
                    out=fill_ps[:],
                    lhsT=w40[:, 0, :, 0],
                    rhs=xq4[:, 0, 0, :],
                    start=True,
                    stop=True,
                )

        copy_names = []
        trig_specs = []  # (trigger ins name, gating copy index)
        fills = _FILLS
        for c in range(nch):
            w4 = wts[c][:].rearrange("p (f i e) -> p f i e", f=KT2, i=P, e=2)
            psT = psum.tile([P, B], f32, tag="psT8")
            for k in range(2 * KT2):
                f, e = k // 2, k % 2
                nc.tensor.matmul(
                    out=psT[:],
                    lhsT=w4[:, f, :, e],
                    rhs=xq4[:, f, e, :],
                    start=(k == 0),
                    stop=(k == 2 * KT2 - 1),
                )
                if c == nch - 1 and k == 19:
                    _fill(fills[3])
            if c == nch - 1:
                # DVE: faster PSUM access than Act, and idle this late
                cp = nc.vector.tensor_copy(obs[:, c, :B], psT[:])
            else:
                cp = nc.scalar.copy(obs[:, c, :B], psT[:])
            copy_names.append(cp.ins.name)
            if c < nch - 1:
                _fill(fills[c])
            if c == nch - 2:
                tr = nc.gpsimd.trigger_dma(count=None, queue_num=1).wait_op(
                    cp_sem, 0, "sem-ge"
                )
                # ordering-only edge: keep prepB's desc-gen (Pool engine)
                # scheduled before this trigger so it is off the tail
                from concourse.bass import InstructionNameOrderedSet

                _dep = InstructionNameOrderedSet()
                _dep.add(prepB.ins.name)
                tr.ins.add_nosync_dependencies_from(_dep)
                trig_specs.append((tr.ins.name, c))
        tr = nc.gpsimd.trigger_dma(count=None, queue_num=2).wait_op(
            cp_sem, 0, "sem-ge"
        )
        trig_specs.append((tr.ins.name, nch - 1))

    # Post-schedule patches. (a) Tile's deferred-dep machinery doesn't bind
    # obs-copy producers emitted after the preps, so the triggers carry
    # placeholder waits: re-point them at the gating copy's engine-tick sem
    # (value = cumulative tick at that copy), which fires at Act ENGINE
    # completion. (b) The Tile pass assigns the preps DMASW lanes and emits
    # teardown waits on them, but a gen_mode==1 prep's transfer bumps only the
    # descriptor-baked user sem -- the lane sems are never incremented
    # (framework gap). Re-point those orphaned waits at the user DMA sems.
    produced = set()
    all_waits = []
    insts = []
    for blk in nc.m.functions[0].blocks:
        for ins in blk.instructions:
            insts.append(ins)
            si = ins.sync_info
            if not si:
                continue
            for u in si.on_update or []:
                produced.add(u.id)
            for w in si.on_wait or []:
                all_waits.append((ins, w))

    # (a0) idx waits: Tile's internal sim can't see the pre-context idx DMA,
    # so the waits were emitted trivially satisfiable; restore the real value
    for ins, w in all_waits:
        if w.ant_name in ("idx_dma", "idx2_dma") and w.wait_value == 0:
            w.wait_value = 16
            si = ins.sync_info
            si.on_wait = si.on_wait

    # (a) trigger gating: find each copy's engine-tick sem + cumulative value
    by_name = {ins.name: ins for ins in insts}
    copy_sem = {}
    for cn in copy_names:
        cp_ins = by_name[cn]
        ups = [
            u
            for u in (cp_ins.sync_info.on_update or [])
            if u.update_mode == "sem-inc"
        ]
        assert len(ups) == 1, f"copy {cn} tick updates: {ups}"
        copy_sem[cn] = (ups[0].id, ups[0].ant_name)
    ticks = {}
    copy_tick = {}
    for ins in insts:
        si = ins.sync_info
        if si:
            for u in si.on_update or []:
                if u.update_mode == "sem-inc":
                    ticks[u.id] = ticks.get(u.id, 0) + u.update_value
        if ins.name in copy_names:
            copy_tick[ins.name] = ticks.get(copy_sem[ins.name][0], 0)
    for trig_name, c in trig_specs:
        trig = by_name[trig_name]
        si = trig.sync_info
        wl = si.on_wait
        patched = 0
        for w in wl:
            if w.ant_name == "cp_done":
                cn = copy_names[c]
                w.id = copy_sem[cn][0]
                w.wait_value = copy_tick[cn]
                try:
                    w.ant_name = copy_sem[cn][1]
                except Exception:
                    pass
                patched += 1
        assert patched == 1, f"trigger {trig_name}: {patched} placeholder waits"
        si.on_wait = wl
    # the scheduler also emits standalone EventSemaphore pre-waits that
    # duplicate the trigger's own (patched) wait; each costs SEM_DELAY on
    # the tail -- neutralize any wait still naming the placeholder sem
    for ins, w in all_waits:
        if w.ant_name == "cp_done":
            si = ins.sync_info
            wl = si.on_wait
            for w2 in wl:
                if w2.ant_name == "cp_done":
                    w2.wait_value = 0
            si.on_wait = wl
    # emulate pass-1's round-robin DMASW lane assignment to map each
    # gen_mode==1 prep to its (orphaned) lane sem, then re-point every wait
    # on that lane at the prep's user DMA sem
    swdge_types = ("InstDMAGatherAnt", "InstDMAScatterAddAnt")
    lane, n_sw = 0, 0
    lane_to_user = {}
    for ins in insts:
        if (
            str(ins.engine) == "EngineType.Pool"
            and type(ins).__name__ in swdge_types
        ):
            this_lane, lane, n_sw = lane, (lane + 1) % 8, n_sw + 1
            if getattr(ins, "gen_mode", 0) == 1:
                u0 = (ins.sync_info.on_update or [None])[0]
                assert u0 is not None and u0.update_value == 16, str(ins)
                lane_to_user[f"DMASW{this_lane}_"] = (u0.id, u0.ant_name)
    assert n_sw <= 8, f"{n_sw} SWDGE DMAs wrap the 8 DMASW lanes"
    for ins, w in all_waits:
        nm = w.ant_name or ""
        hit = [v for pref, v in lane_to_user.items() if nm.startswith(pref)]
        if hit:
            assert w.wait_value == 16, f"{ins.name}: {w}"
            si = ins.sync_info
            wl = si.on_wait
            for w2 in wl:
                if w2.ant_name == nm:
                    w2.id = hit[0][0]
                    try:
                        w2.ant_name = hit[0][1]
                    except Exception:
                        pass
            si.on_wait = wl

    # (c) hoist the pre-context idx DMAs (and their sem clears) ahead of the
    # Bacc start barrier in the entry block: the HWDGE chain needs no
    # cross-engine init, so the idx transfer starts at t~0 instead of ~616,
    # pulling the whole gather pipeline earlier.
    blk0 = nc.m.functions[0].blocks[0]
    b_ins = blk0.instructions
    head = [x for x in b_ins if type(x).__name__ == "InstCall"]
    clears = [
        x
        for x in b_ins
        if type(x).__name__ == "InstISA" and str(x.engine).endswith("Pool")
    ]
    dmas = [x for x in b_ins if type(x).__name__ == "InstDMACopy"]
    moved = set(id(x) for x in head + clears + dmas)
    rest = [x for x in b_ins if id(x) not in moved]
    assert len(head) == 1 and len(clears) == 2 and len(dmas) == 2, (
        len(head),
        len(clears),
        len(dmas),
    )
    blk0.instructions = head + clears + dmas + rest

    nc.compile()
    _cache[key] = nc
    return nc


def _wrap_idx(idx_pad):
    """[npad] int16 -> [128, npad//16] wrapped-16 layout, replicated 8x."""
    npad = idx_pad.shape[0]
    blk = idx_pad.reshape(npad // 16, 16).T  # [16, npad//16]
    return np.ascontiguousarray(np.tile(blk, (8, 1)))


def _make_in_maps_hi8(x, weight, indices):
    """Host prep for the fp8 kernel: dedup+shard indices, e3m4 cast.

    Returns (in_maps, assemble_fn) or None if the shape doesn't fit."""
    import ml_dtypes

    x = np.asarray(x, dtype=np.float32)
    weight = np.asarray(weight, dtype=np.float32)
    indices = np.asarray(indices, dtype=np.int64)

    uniq, inv = np.unique(indices, return_inverse=True)
    nu = len(uniq)
    base, rem = divmod(nu, NCORES)
    counts = [base + (1 if c < rem else 0) for c in range(NCORES)]
    # scatter B statically covers rows 384..463 only
    if max(counts) > NV8 - 128 + 80:
        return None
    starts = np.concatenate([[0], np.cumsum(counts)[:-1]])

    whi = np.ascontiguousarray(
        np.clip(weight * W_SCALE, -15.5, 15.5).astype(ml_dtypes.float8_e3m4)
    )
    # xq[p, f, e, b] = fp16(x[b, 0, 2*(f*128+p)+e] / W_SCALE)
    xt = x[:, 0, :].T / W_SCALE               # [4096, B]
    xq = np.ascontiguousarray(
        xt.reshape(KT2, P, 2, B).transpose(1, 0, 2, 3).reshape(P, KT2 * 2 * B)
    ).astype(np.float16)

    # scatter iota: rows 0..383 for scatter A, then slice-relative 0..127
    # for scatter B (its out_ap starts at row 384)
    iota = _wrap_idx(
        np.concatenate(
            [np.arange(NV8 - 128, dtype=np.int16), np.arange(128, dtype=np.int16)]
        )
    )
    in_maps = []
    for c in range(NCORES):
        idx_pad = np.full(NV8, uniq[starts[c]], dtype=np.int16)
        idx_pad[: counts[c]] = uniq[starts[c] : starts[c] + counts[c]]
        idx_full = np.concatenate([_wrap_idx(idx_pad), iota], axis=1)
        in_maps.append(
            {"whi": whi, "xq": xq, "idx": np.ascontiguousarray(idx_full)}
        )

    def assemble(results):
        cols = np.empty((B, nu), dtype=np.float32)
        for c in range(NCORES):
            cols[:, starts[c] : starts[c] + counts[c]] = (
                results[c]["out"][: counts[c], :B].T
            )
        return np.ascontiguousarray(cols[:, inv].reshape(B, 1, R_TOTAL))

    return in_maps, assemble


def _build_hi(nvalid, gbufs=3, reps=1):
    """fp16-only weight-stationary kernel (fallback path)."""
    key = ("hi5", nvalid, gbufs, reps)
    if key in _cache:
        return _cache[key]
    from concourse import bacc, mybir, tile

    f32 = mybir.dt.float32
    f16 = mybir.dt.float16
    i16 = mybir.dt.int16

    nfull = nvalid // 128      # full 128-row transpose-gather chunks
    nt = nvalid % 128          # odd-size tail chunk, non-transpose gather
    nch = nfull + (1 if nt else 0)
    nv16 = nvalid // 16

    nc = bacc.Bacc(
        "TRN2",
        target_bir_lowering=False,
        debug=False,
        enable_asserts=False,
        num_swdge_queues=2,
    )
    whi_dram = nc.dram_tensor("whi", [D_FF, D_MODEL], f16, kind="ExternalInput").ap()
    xh_dram = nc.dram_tensor("xh", [P, KT * B], f16, kind="ExternalInput").ap()
    if nt:
        id_dram = nc.dram_tensor("ident", [nt, nt], f16, kind="ExternalInput").ap()
    idx_dram = nc.dram_tensor("idx", [P, 2 * nv16], i16, kind="ExternalInput").ap()
    out_dram = nc.dram_tensor("out", [nvalid, 2 * B], f32, kind="ExternalOutput").ap()

    with tile.TileContext(nc) as tc, ExitStack() as ctx:
        consts = ctx.enter_context(tc.tile_pool(name="consts", bufs=1))
        whi_pool = ctx.enter_context(tc.tile_pool(name="whiT", bufs=max(gbufs, nch)))
        psum = ctx.enter_context(tc.tile_pool(name="psum", bufs=4, space="PSUM"))

        idx_sb = consts.tile([P, 2 * nv16], i16)
        nc.sync.dma_start(idx_sb[:], idx_dram)
        xh_sb = consts.tile([P, KT * B], f16)
        nc.sync.dma_start(xh_sb[:], xh_dram)
        obs = consts.tile([P, nch, 2 * B], f32)
        nc.vector.memset(obs[:], 0)
        if nt:
            id_sb = consts.tile([nt, nt], f16)
            nc.sync.dma_start(id_sb[:], id_dram)
            w4 = consts.tile([P, 1, D_MODEL], f16)

        whiTs = []
        if nt:
            nc.gpsimd.dma_gather(
                out_ap=w4[:],
                in_ap=whi_dram,
                idxs_ap=idx_sb[:, nfull * 8 : nv16],
                num_idxs=nt,
                num_idxs_reg=nt,
                elem_size=D_MODEL,
                transpose=False,
            )
        for c in range(nfull):
            r0 = c * 128
            whiT = whi_pool.tile([P, KT, 128], f16, tag=f"whiT{c}")
            if c == nfull - 1:
                for h in range(4):
                    nc.gpsimd.dma_gather(
                        out_ap=whiT[:, h * 8 : (h + 1) * 8, :],
                        in_ap=whi_dram[:, h * 1024 : (h + 1) * 1024],
                        idxs_ap=idx_sb[:, r0 // 16 : r0 // 16 + 8],
                        num_idxs=128,
                        num_idxs_reg=128,
                        elem_size=1024,
                        elem_step=D_MODEL,
                        transpose=True,
                    )
            else:
                nc.gpsimd.dma_gather(
                    out_ap=whiT[:],
                    in_ap=whi_dram,
                    idxs_ap=idx_sb[:, r0 // 16 : r0 // 16 + 8],
                    num_idxs=128,
                    num_idxs_reg=128,
                    elem_size=D_MODEL,
                    transpose=True,
                )
            whiTs.append(whiT)
        if nt:
            whiT_nt = whi_pool.tile([P, KT, nt], f16, tag="whiTnt")
            for k in range(KT):
                psX = psum.tile([P, nt], f16, tag="psX")
                nc.tensor.transpose(
                    psX[:], w4[:nt, 0, k * P : (k + 1) * P], id_sb[:]
                )
                nc.vector.tensor_copy(whiT_nt[:, k, :], psX[:])
            whiTs.append(whiT_nt)

        order = list(range(nch))
        if nt and nch >= 2:
            order = order[: nch - 2] + [nch - 1, nch - 2]
        for c in order:
            whiT = whiTs[c]
            rows = nt if (nt and c == nch - 1) else 128
            r0 = nfull * 128 if (nt and c == nch - 1) else c * 128
            psT = psum.tile([rows, B], f32, tag="psT")
            for k in range(KT):
                nc.tensor.matmul(
                    out=psT[:],
                    lhsT=whiT[:, k, :],
                    rhs=xh_sb[:, k * B : (k + 1) * B],
                    start=(k == 0),
                    stop=(k == KT - 1),
                )
            nc.scalar.copy(obs[:rows, c, :B], psT[:])
            nc.sync.dma_start(out_dram[r0 : r0 + rows, :B], obs[:rows, c, :B])

    nc.compile()
    _cache[key] = nc
    return nc


def _make_in_maps_hi(x, weight, indices):
    """Host prep for the fp16 fallback kernel."""
    x = np.asarray(x, dtype=np.float32)
    weight = np.asarray(weight, dtype=np.float32)
    indices = np.asarray(indices, dtype=np.int64)

    whi = np.ascontiguousarray(weight.astype(np.float16))
    xt = np.ascontiguousarray(
        x[:, 0, :].reshape(B, KT, P).transpose(2, 1, 0).reshape(P, KT * B)
    )
    xh = np.ascontiguousarray(xt.astype(np.float16))

    uniq, inv = np.unique(indices, return_inverse=True)
    nu = len(uniq)
    base, rem = divmod(nu, NCORES)
    counts = [base + (1 if c < rem else 0) for c in range(NCORES)]
    starts = np.concatenate([[0], np.cumsum(counts)[:-1]])
    nvalid = -(-max(counts) // 16) * 16

    iota = _wrap_idx(np.arange(nvalid, dtype=np.int16))
    ident = np.eye(nvalid % 128 or 1, dtype=np.float16)
    in_maps = []
    for c in range(NCORES):
        idx_pad = np.zeros(nvalid, dtype=np.int16)
        idx_pad[: counts[c]] = uniq[starts[c] : starts[c] + counts[c]]
        idx_full = np.concatenate([_wrap_idx(idx_pad), iota], axis=1)
        in_maps.append(
            {"whi": whi, "xh": xh, "idx": np.ascontiguousarray(idx_full),
             "ident": ident}
        )

    def assemble(results):
        cols = np.empty((B, nu), dtype=np.float32)
        for c in range(NCORES):
            cols[:, starts[c] : starts[c] + counts[c]] = (
                results[c]["out"][: counts[c], :B].T
            )
        return np.ascontiguousarray(cols[:, inv].reshape(B, 1, R_TOTAL))

    return in_maps, assemble, nvalid


def _filter_in_maps(nc, in_maps):
    names = set()
    from concourse import mybir

    for alloc in nc.m.functions[0].allocations:
        if isinstance(alloc, mybir.MemoryLocationSet) and alloc.kind == "ExternalInput":
            names.add(alloc.memorylocations[0].name)
    return [{k: v for k, v in m.items() if k in names} for m in in_maps]


def run_full(x, weight, indices, trace=False, mode="hi8"):
    """Returns (output, BassKernelResults)."""
    from concourse.bass_utils import run_bass_kernel_spmd

    if mode == "hi8":
        prep = _make_in_maps_hi8(x, weight, indices)
        if prep is not None:
            in_maps, assemble = prep
            nc = _build_hi8()
            in_maps = _filter_in_maps(nc, in_maps)
            res = run_bass_kernel_spmd(nc, in_maps, list(range(NCORES)), trace=trace)
            return assemble(res.results), res
        mode = "hi"

    in_maps, assemble, nvalid = _make_in_maps_hi(x, weight, indices)
    nc = _build_hi(nvalid)
    in_maps = _filter_in_maps(nc, in_maps)
    res = run_bass_kernel_spmd(nc, in_maps, list(range(NCORES)), trace=trace)
    return assemble(res.results), res


def kernel(x, weight, indices):
    out, _ = run_full(x, weight, indices)
    return out
